# revision 1
# baseline (speedup 1.0000x reference)
"""Trainium2 Bass kernel for nn_ConvolutionalNMPBlock.

Self-contained: takes full (unsharded) inputs, shards batch across 8
NeuronCores (2 elements each), runs a fused Bass/Tile kernel, gathers.
"""
import numpy as np
import ml_dtypes

BS, N, D = 16, 2048, 256
NCORE = 8
PER = BS // NCORE          # batch elements per core
EPS = 1e-5
NB = N // 128              # 16 row blocks
LC = N // 512              # 4 column chunks of 512
DC = D // 128              # 2 channel blocks
KT = 17                    # conv2 taps
BF = ml_dtypes.bfloat16

_built = {}                # use_mask -> compiled nc


def _build(use_mask: bool, use_bias: bool = True, loop_n: int = 1,
           skip: frozenset = frozenset(), body_reps: int = 1):
    from concourse import bacc, tile
    import concourse.mybir as mybir
    from contextlib import ExitStack

    f32 = mybir.dt.float32
    bf16 = mybir.dt.bfloat16
    AF = mybir.ActivationFunctionType
    OP = mybir.AluOpType

    nc = bacc.Bacc("TRN2", target_bir_lowering=False, debug=False,
                   num_devices=NCORE)

    def din(name, shape, dt=f32):
        return nc.dram_tensor(name, shape, dt, kind="ExternalInput").ap()

    x_d = din("x", (PER, N, D))
    w1_d = din("w1t", (128, DC, DC, 128), bf16)        # [p=kin, kc, mc, m]
    w2_d = din("w2t", (128, DC, KT, DC, 128), bf16)    # [p, kc, tap, mc, m]
    sh1_d = din("sh1", (128, DC))
    sh2_d = din("sh2", (128, DC))
    wmsg_d = din("wmsgt", (128, DC, D), bf16)          # [p, kc, f]
    bmsg_d = din("bmsg", (1, D), bf16)
    wseh_d = din("wseth", (128, DC, 3), bf16)
    wsel_d = din("wsetl", (128, DC, 3), bf16)
    bse_d = din("bse", (1, 3), bf16)
    wih_d = din("wiht", (128, 4, 3 * D), bf16)         # [p, kc, f]
    whh_d = din("whht", (128, DC, 3 * D), bf16)
    brz_d = din("brow_rz", (1, 2 * D), bf16)
    bgin_d = din("brow_gin", (1, D), bf16)
    bghn_d = din("brow_ghn", (1, D), bf16)
    ones_d = din("ones128", (1, 128), bf16)
    ones5_d = din("ones512", (1, 512), bf16)
    ones2n_d = din("ones2n", (2, N), bf16)
    if use_mask:
        mt_d = din("maskt", (PER, N, N), bf16)
    out_d = nc.dram_tensor("out", (PER, N, D), f32, kind="ExternalOutput").ap()

    with tile.TileContext(nc) as tc, ExitStack() as ctx:
        if loop_n > 1:
            ctx.enter_context(tc.For_i(0, loop_n, 1))
        wp = ctx.enter_context(tc.tile_pool(name="wp", bufs=1))
        big = ctx.enter_context(tc.tile_pool(name="big", bufs=1))
        rawp = ctx.enter_context(tc.tile_pool(name="rawp", bufs=1))
        atp = ctx.enter_context(tc.tile_pool(name="atp", bufs=33))
        cvp = ctx.enter_context(tc.tile_pool(name="cvp", bufs=2))
        gtp = ctx.enter_context(tc.tile_pool(name="gtp", bufs=1))
        dramp = ctx.enter_context(tc.tile_pool(name="dramp", bufs=1, space="DRAM"))
        ps = ctx.enter_context(tc.tile_pool(name="ps", bufs=4, space="PSUM"))
        pa = ctx.enter_context(tc.tile_pool(name="pa", bufs=2, space="PSUM"))

        # ---- load weights (persistent) ----
        w1 = wp.tile([128, DC, DC, 128], bf16, tag="w1")
        nc.gpsimd.dma_start(w1[:], w1_d[:])
        sh1 = wp.tile([128, DC], f32, tag="sh1")
        nc.gpsimd.dma_start(sh1[:], sh1_d[:])
        sh2 = wp.tile([128, DC], f32, tag="sh2")
        nc.gpsimd.dma_start(sh2[:], sh2_d[:])
        bmsg = wp.tile([1, D], bf16, tag="bmsg")
        nc.gpsimd.dma_start(bmsg[:], bmsg_d[:])
        wseh = wp.tile([128, DC, 3], bf16, tag="wseh")
        nc.gpsimd.dma_start(wseh[:], wseh_d[:])
        wsel = wp.tile([128, DC, 3], bf16, tag="wsel")
        nc.gpsimd.dma_start(wsel[:], wsel_d[:])
        bse = wp.tile([1, 3], bf16, tag="bse")
        nc.gpsimd.dma_start(bse[:], bse_d[:])
        ones = wp.tile([1, 128], bf16, tag="ones")
        nc.gpsimd.dma_start(ones[:], ones_d[:])
        ones5 = wp.tile([1, 512], bf16, tag="ones5")
        nc.gpsimd.dma_start(ones5[:], ones5_d[:])

        for el in [e for _ in range(body_reps) for e in range(PER)]:
            # ---- load x n-major (contiguous), split bf16 hi/lo, and build
            # channel-major copies via the XBAR transpose DMA (2-byte dtype).
            xnf = rawp.tile([128, NB, D], f32, tag="xnf", bufs=2)
            xnh = rawp.tile([128, NB, D], bf16, tag="xnh", bufs=1)
            xnl = rawp.tile([128, NB, D], bf16, tag="xnl", bufs=1)
            # layout: xth[p_c, nb, dc, p_n] == xT[dc*128+p_c, nb*128+p_n]
            xth = big.tile([128, NB, DC, 128], bf16, tag="xth", bufs=2)
            xtl = big.tile([128, NB, DC, 128], bf16, tag="xtl", bufs=1)
            qn = NB // 4
            quarters = [slice(h * qn, (h + 1) * qn) for h in range(4)]
            for h_, hs in enumerate(quarters):
                nc.sync.dma_start(
                    xnf[:, hs, :],
                    x_d[el, h_ * (N // 4):(h_ + 1) * (N // 4), :].rearrange(
                        "(nb p) d -> p nb d", p=128))
            for hs in quarters:
                nc.vector.tensor_copy(xnh[:, hs, :], xnf[:, hs, :])
                nc.vector.scalar_tensor_tensor(xnl[:, hs, :], xnf[:, hs, :],
                                               1.0, xnh[:, hs, :],
                                               OP.mult, OP.subtract)
            for hs in quarters:
                nc.scalar.dma_start_transpose(
                    xth[:, hs].rearrange("p nb dc pn -> p (nb dc) pn"),
                    xnh[:, hs, :])
            for hs in quarters:
                nc.scalar.dma_start_transpose(
                    xtl[:, hs].rearrange("p nb dc pn -> p (nb dc) pn"),
                    xnl[:, hs, :])

            if el == 0:
                w2 = wp.tile([128, DC, KT, DC, 128], bf16, tag="w2")
                nc.sync.dma_start(w2[:], w2_d[:])
                wmsg = wp.tile([128, DC, D], bf16, tag="wmsg")
                nc.sync.dma_start(wmsg[:], wmsg_d[:])
                wih = wp.tile([128, 4, 3 * D], bf16, tag="wih")
                nc.sync.dma_start(wih[:], wih_d[:])
                whh = wp.tile([128, DC, 3 * D], bf16, tag="whh")
                nc.sync.dma_start(whh[:], whh_d[:])
                brz = wp.tile([1, 2 * D], bf16, tag="brz")
                nc.sync.dma_start(brz[:], brz_d[:])
                bgin = wp.tile([1, D], bf16, tag="bgin")
                nc.sync.dma_start(bgin[:], bgin_d[:])
                bghn = wp.tile([1, D], bf16, tag="bghn")
                nc.sync.dma_start(bghn[:], bghn_d[:])
            # ---- s = x @ w_se.T + b_se  (f32-accurate via hi/lo) ----
            st = big.tile([3, N], f32, tag="st", bufs=1)
            for c in range(LC):
                pss = ps.tile([3, 512], f32, tag="ps")
                first = True
                for kc in range(DC):
                    for wi_, (wse_, xt_) in enumerate(
                            ((wseh, xth), (wseh, xtl), (wsel, xth))):
                        last = (not use_bias) and kc == DC - 1 and wi_ == 2
                        nc.tensor.matmul(pss[:], wse_[:, kc, :],
                                         xt_[:, 4 * c:4 * (c + 1), kc, :],
                                         start=first, stop=last,
                                         skip_group_check=True)
                        first = False
                if use_bias:
                    nc.tensor.matmul(pss[:], bse[:], ones5[:], start=False,
                                     stop=True, skip_group_check=True)
                nc.vector.tensor_copy(st[:, c * 512:(c + 1) * 512], pss[:])

            # ---- augmented 13-row factors for exp(-dist) matmul ----
            # k-row pairs (SA | SB): 0:3 (2s_hi | s_hi), 3:6 (2s_lo | s_hi),
            # 6:9 (2s_hi | s_lo), 9,10 (1 | -sq_hi, -sq_lo),
            # 11,12 (-sq_hi, -sq_lo | 1).
            # Built in a WIDE (128-partition) layout: st (3, N) costs ~2k DVE
            # cycles per op on 3 lanes, so bounce to (128, 3, NB) via DRAM,
            # do all the arithmetic at 128-lane width, and bounce the
            # finished 13-row stacks back.  w[p, r, g] == row[r, g*128+p].
            scst = dramp.tile([3, N], f32, tag="scst", bufs=1)
            nc.sync.dma_start(scst[:], st[:])
            stw = cvp.tile([128, 3, NB], f32, tag="stw", bufs=1)
            nc.sync.dma_start(stw[:], scst[:].rearrange("c (g p) -> p c g", p=128))
            ssqw = cvp.tile([128, 3, NB], f32, tag="ssqw", bufs=1)
            nc.scalar.activation(ssqw[:], stw[:], AF.Square)
            sqw = cvp.tile([128, NB], f32, tag="sqw", bufs=1)
            nc.vector.tensor_tensor(sqw[:], ssqw[:, 0, :], ssqw[:, 1, :], OP.add)
            nc.vector.tensor_tensor(sqw[:], sqw[:], ssqw[:, 2, :], OP.add)

            saw = cvp.tile([128, 13, NB], bf16, tag="saw", bufs=1)
            sbw = cvp.tile([128, 13, NB], bf16, tag="sbw", bufs=1)
            nc.vector.tensor_scalar(saw[:, 0:3, :], stw[:], 2.0, None, OP.mult)
            nc.vector.scalar_tensor_tensor(saw[:, 3:6, :], stw[:], 2.0,
                                           saw[:, 0:3, :], OP.mult, OP.subtract)
            nc.vector.tensor_copy(saw[:, 6:9, :], saw[:, 0:3, :])
            nc.vector.memset(saw[:, 9:11, :], 1.0)
            nc.vector.tensor_scalar(saw[:, 11:12, :], sqw[:].unsqueeze(1),
                                    -1.0, None, OP.mult)
            nc.vector.scalar_tensor_tensor(saw[:, 12:13, :], sqw[:].unsqueeze(1),
                                           -1.0, saw[:, 11:12, :],
                                           OP.mult, OP.subtract)
            nc.vector.tensor_copy(sbw[:, 0:3, :], stw[:])
            nc.vector.tensor_copy(sbw[:, 3:6, :], sbw[:, 0:3, :])
            nc.vector.scalar_tensor_tensor(sbw[:, 6:9, :], stw[:], 1.0,
                                           sbw[:, 0:3, :], OP.mult, OP.subtract)
            nc.vector.tensor_copy(sbw[:, 9:11, :], saw[:, 11:13, :])
            nc.vector.memset(sbw[:, 11:13, :], 1.0)

            sa = big.tile([13, N], bf16, tag="sa", bufs=1)
            sb = big.tile([13, N], bf16, tag="sb", bufs=1)
            scsa = dramp.tile([13, N], bf16, tag="scsa", bufs=1)
            scsb = dramp.tile([13, N], bf16, tag="scsb", bufs=1)
            nc.sync.dma_start(scsa[:].rearrange("r (g p) -> p r g", p=128), saw[:])
            nc.sync.dma_start(sa[:], scsa[:])
            nc.sync.dma_start(scsb[:].rearrange("r (g p) -> p r g", p=128), sbw[:])
            nc.sync.dma_start(sb[:], scsb[:])

            # ---- conv1 (1x1) + bn1 + relu -> h1 (padded by 8 each side) ----
            h1 = big.tile([128, DC, N + 16], bf16, tag="h1", bufs=1)
            nc.vector.memset(h1[:, :, 0:8], 0.0)
            nc.vector.memset(h1[:, :, N + 8:N + 16], 0.0)
            for mc in range(DC):
                for c in range(LC):
                    pc = ps.tile([128, 512], f32, tag="ps")
                    for kc in range(DC):
                        nc.tensor.matmul(pc[:], w1[:, kc, mc, :],
                                         xth[:, 4 * c:4 * (c + 1), kc, :],
                                         start=(kc == 0), stop=(kc == DC - 1),
                                         skip_group_check=True)
                    nc.scalar.activation(h1[:, mc, 8 + c * 512:8 + (c + 1) * 512],
                                         pc[:], AF.Relu, bias=sh1[:, mc:mc + 1])

            # ---- conv2 (17 taps) + bn2 + residual + relu -> x_convT ----
            xcv = big.tile([128, DC, N], bf16, tag="xcv")
            if "conv2" in skip:
                nc.vector.memset(xcv[:], 0.0)
            for mc in range(0 if "conv2" in skip else DC):
                for c in range(LC):
                    pc2 = ps.tile([128, 512], f32, tag="ps")
                    first = True
                    for kc in range(DC):
                        for t in range(KT):
                            nc.tensor.matmul(
                                pc2[:], w2[:, kc, t, mc, :],
                                h1[:, kc, c * 512 + t:c * 512 + t + 512],
                                start=first, stop=(kc == DC - 1 and t == KT - 1),
                                skip_group_check=True)
                            first = False
                    tv = cvp.tile([128, 4, 128], f32, tag="cv", bufs=2)
                    nc.vector.tensor_tensor(tv[:], pc2[:].rearrange(
                        "p (a b) -> p a b", b=128),
                        xth[:, 4 * c:4 * (c + 1), mc, :], OP.add)
                    nc.vector.tensor_tensor(tv[:], tv[:],
                                            xtl[:, 4 * c:4 * (c + 1), mc, :],
                                            OP.add)
                    nc.scalar.activation(
                        xcv[:, mc, c * 512:(c + 1) * 512],
                        tv[:].rearrange("p a b -> p (a b)"),
                        AF.Relu, bias=sh2[:, mc:mc + 1])

            # ---- msg = relu(x @ w_msg.T + b_msg), n-major ----
            msg = big.tile([128, NB, D], bf16, tag="msg")
            for nb in range(NB):
                pm = ps.tile([128, 512], f32, tag="ps")
                for kc in range(DC):
                    nc.tensor.matmul(pm[:, 0:D], xth[:, nb, kc, :],
                                     wmsg[:, kc, :], start=(kc == 0),
                                     stop=(not use_bias and kc == DC - 1),
                                     skip_group_check=True)
                if use_bias:
                    nc.tensor.matmul(pm[:, 0:D], ones[:], bmsg[:], start=False,
                                     stop=True, skip_group_check=True)
                nc.scalar.activation(msg[:, nb, :], pm[:, 0:D], AF.Relu)

            # ---- A-branch: x_nmpT[d, i] = sum_j exp(-dist[j,i]) * msg[j, d] ----
            xnm = big.tile([128, DC, N], bf16, tag="xnm")
            if "noG" in skip and el == 0:
                atc = wp.tile([128, 512], bf16, tag="atc")
                nc.vector.memset(atc[:], 0.001)
            if "A" in skip:
                nc.vector.memset(xnm[:], 0.0)

            # A-branch restructured into long clean matmul runs (conv2-style):
            # per i-chunk, batch all 16 G matmuls + exps, then run the 16
            # m0-accumulations back-to-back into one bank, then the 16 m1 —
            # software-pipelined one chunk deep (acc of chunk ic-1 overlaps
            # exp of chunk ic).  Interleaved short groups measured ~560 ns/MM
            # on HW vs ~240 ns/MM for long runs.
            at_store = {}

            def emit_acc(ic):
                accs = [pa.tile([128, 512], f32, tag="acc0", name="a0"),
                        pa.tile([128, 512], f32, tag="acc1", name="a1")]
                for mc in range(DC):
                    for jb in range(NB):
                        nc.tensor.matmul(accs[mc][:],
                                         msg[:, jb, mc * 128:(mc + 1) * 128],
                                         at_store[(ic, jb)][:],
                                         start=(jb == 0), stop=(jb == NB - 1),
                                         skip_group_check=True)
                for mc in range(DC):
                    nc.vector.tensor_copy(xnm[:, mc, ic * 512:(ic + 1) * 512],
                                          accs[mc][:])

            for ic in range(0 if "A" in skip else LC):
                for jb in range(NB):
                    if "noG" in skip:
                        at_store[(ic, jb)] = atc
                        continue
                    pgm = ps.tile([128, 512], f32, tag="ps")
                    nc.tensor.matmul(pgm[:], sa[:, jb * 128:(jb + 1) * 128],
                                     sb[:, ic * 512:(ic + 1) * 512],
                                     start=True, stop=True,
                                     skip_group_check=True)
                    at = atp.tile([128, 512], bf16, tag="at")
                    if "exp2dve" in skip:
                        nc.vector.tensor_copy(at[:], pgm[:])
                    else:
                        nc.scalar.activation(at[:], pgm[:], AF.Exp)
                    if use_mask:
                        mtt = cvp.tile([128, 512], bf16, tag="mtt")
                        nc.sync.dma_start(mtt[:],
                                          mt_d[el, jb * 128:(jb + 1) * 128,
                                               ic * 512:(ic + 1) * 512])
                        nc.vector.tensor_tensor(at[:], at[:], mtt[:], OP.mult)
                    at_store[(ic, jb)] = at
                if ic >= 1:
                    emit_acc(ic - 1)
            if "A" not in skip:
                emit_acc(LC - 1)

            # ---- GRU gates (n-major) ----
            for nb in range(NB):
                sl = slice(nb * 128, (nb + 1) * 128)
                prz = ps.tile([128, 512], f32, tag="ps")
                ih_srcs = [xcv[:, 0, sl], xcv[:, 1, sl], xnm[:, 0, sl], xnm[:, 1, sl]]
                hh_srcs = [xth[:, nb, 0, :], xth[:, nb, 1, :]]
                for ci in range(4):
                    nc.tensor.matmul(prz[:], ih_srcs[ci], wih[:, ci, 0:512],
                                     start=(ci == 0), stop=False,
                                     skip_group_check=True)
                for kc in range(DC):
                    nc.tensor.matmul(prz[:], hh_srcs[kc], whh[:, kc, 0:512],
                                     start=False,
                                     stop=(not use_bias and kc == DC - 1),
                                     skip_group_check=True)
                if use_bias:
                    nc.tensor.matmul(prz[:], ones[:], brz[:], start=False,
                                     stop=True, skip_group_check=True)

                # gi_n in cols 0:D, gh_n in cols D:2D of ONE psum tile —
                # 2 psum allocs per block instead of 3, so two blocks pipeline
                pgg = ps.tile([128, 512], f32, tag="ps")
                for ci in range(4):
                    nc.tensor.matmul(pgg[:, 0:D], ih_srcs[ci], wih[:, ci, 512:768],
                                     start=(ci == 0),
                                     stop=(not use_bias and ci == 3),
                                     skip_group_check=True)
                if use_bias:
                    nc.tensor.matmul(pgg[:, 0:D], ones[:], bgin[:], start=False,
                                     stop=True, skip_group_check=True)
                for kc in range(DC):
                    nc.tensor.matmul(pgg[:, D:2 * D], hh_srcs[kc],
                                     whh[:, kc, 512:768], start=(kc == 0),
                                     stop=(not use_bias and kc == DC - 1),
                                     skip_group_check=True)
                if use_bias:
                    nc.tensor.matmul(pgg[:, D:2 * D], ones[:], bghn[:],
                                     start=False, stop=True,
                                     skip_group_check=True)

                tr = gtp.tile([128, D], f32, tag="tr")
                nc.scalar.activation(tr[:], prz[:, 0:D], AF.Tanh, scale=0.5)
                tz = gtp.tile([128, D], f32, tag="tz")
                nc.scalar.activation(tz[:], prz[:, D:2 * D], AF.Tanh, scale=0.5)
                # r,z in place; q accumulates in place; ee overwrites dd
                nc.vector.tensor_scalar(tz[:], tz[:], 0.5, 0.5, OP.mult, OP.add)
                nc.vector.tensor_scalar(tr[:], tr[:], 0.5, 0.5, OP.mult, OP.add)
                q = gtp.tile([128, D], f32, tag="q")
                nc.vector.tensor_tensor(q[:], tr[:], pgg[:, D:2 * D], OP.mult)
                nc.vector.tensor_tensor(q[:], q[:], pgg[:, 0:D], OP.add)
                nn = gtp.tile([128, D], f32, tag="nn")
                nc.scalar.activation(nn[:], q[:], AF.Tanh)
                dd = gtp.tile([128, D], f32, tag="dd")
                nc.vector.tensor_tensor(dd[:], xnf[:, nb, :], nn[:], OP.subtract)
                nc.vector.tensor_tensor(dd[:], tz[:], dd[:], OP.mult)
                ho = gtp.tile([128, D], f32, tag="ho", bufs=2)
                nc.vector.tensor_tensor(ho[:], nn[:], dd[:], OP.add)
                nc.sync.dma_start(out_d[el, sl, :], ho[:])

    nc.compile()
    return nc


def _host_prep(inputs):
    g = {k: np.asarray(v, np.float32) for k, v in inputs.items()}
    sc1 = g["bn1_g"] / np.sqrt(g["bn1_v"] + EPS)
    sh1 = g["bn1_b"] - g["bn1_m"] * sc1
    sc2 = g["bn2_g"] / np.sqrt(g["bn2_v"] + EPS)
    sh2 = g["bn2_b"] - g["bn2_m"] * sc2

    w1p = g["conv1_w"][:, :, 0] * sc1[:, None]          # (O, I)
    w2p = g["conv2_w"] * sc2[:, None, None]             # (O, I, 17)

    def lhsT_pack(w):   # (O, I) -> (128, kc=I/128, mc=O/128, 128): [p,kc,mc,m]
        o, i = w.shape
        return np.ascontiguousarray(np.transpose(
            w.T.reshape(i // 128, 128, o // 128, 128), (1, 0, 2, 3)))

    w1t = lhsT_pack(w1p).astype(BF)
    w2t = np.stack([lhsT_pack(w2p[:, :, t]) for t in range(KT)], axis=2)
    w2t = np.ascontiguousarray(np.transpose(w2t, (0, 1, 2, 3, 4)))  # [p,kc,t,mc,m]
    w2t = w2t.astype(BF)

    def rhs_pack(wt):   # (Kdim, F) -> (128, kc, F)
        k, f = wt.shape
        return np.ascontiguousarray(
            np.transpose(wt.reshape(k // 128, 128, f), (1, 0, 2)))

    wmsgt = rhs_pack(g["w_msg"].T).astype(BF)
    wiht = rhs_pack(g["w_ih"].T).astype(BF)
    whht = rhs_pack(g["w_hh"].T).astype(BF)

    wse_t = g["w_se"].T                                  # (256, 3)
    wse_hi = wse_t.astype(BF)
    wse_lo = (wse_t - wse_hi.astype(np.float32)).astype(BF)
    wseth = rhs_pack(wse_hi.astype(np.float32)).astype(BF)
    wsetl = rhs_pack(wse_lo.astype(np.float32)).astype(BF)

    bih, bhh = g["b_ih"], g["b_hh"]
    feed = {
        "w1t": w1t, "w2t": w2t,
        "sh1": np.ascontiguousarray(sh1.reshape(DC, 128).T.astype(np.float32)),
        "sh2": np.ascontiguousarray(sh2.reshape(DC, 128).T.astype(np.float32)),
        "wmsgt": wmsgt, "bmsg": g["b_msg"].reshape(1, D).astype(BF),
        "wseth": wseth, "wsetl": wsetl,
        "bse": g["b_se"].reshape(1, 3).astype(BF),
        "wiht": wiht, "whht": whht,
        "brow_rz": (bih[:2 * D] + bhh[:2 * D]).reshape(1, 2 * D).astype(BF),
        "brow_gin": bih[2 * D:].reshape(1, D).astype(BF),
        "brow_ghn": bhh[2 * D:].reshape(1, D).astype(BF),
        "ones128": np.ones((1, 128), BF),
        "ones512": np.ones((1, 512), BF),
        "ones2n": np.ones((2, N), BF),
    }
    return g, feed


def make_in_maps(inputs):
    g, feed = _host_prep(inputs)
    x = g["x"]
    mask = g["mask"]
    use_mask = not bool(np.all(mask == 1.0))
    use_bias = not (np.all(g["b_se"] == 0) and np.all(g["b_msg"] == 0)
                    and np.all(g["b_ih"] == 0) and np.all(g["b_hh"] == 0))
    in_maps = []
    for i in range(NCORE):
        m = dict(feed)
        m["x"] = np.ascontiguousarray(x[i * PER:(i + 1) * PER])
        if use_mask:
            m["maskt"] = np.ascontiguousarray(
                mask[i * PER:(i + 1) * PER].transpose(0, 2, 1)).astype(BF)
        in_maps.append(m)
    return in_maps, use_mask, use_bias


def get_nc(use_mask: bool, use_bias: bool = True):
    key = (use_mask, use_bias)
    if key not in _built:
        _built[key] = _build(use_mask, use_bias)
    return _built[key]


def kernel(**inputs) -> np.ndarray:
    in_maps, use_mask, use_bias = make_in_maps(inputs)
    nc = get_nc(use_mask, use_bias)
    from concourse import bass_utils
    last_err = None
    for attempt in range(3):
        try:
            res = bass_utils.run_bass_kernel_spmd(nc, in_maps,
                                                  core_ids=list(range(NCORE)))
            out = np.concatenate([res.results[i]["out"] for i in range(NCORE)],
                                 axis=0)
            return np.ascontiguousarray(out.astype(np.float32))
        except Exception as e:  # wedged device: reset backend and retry
            last_err = e
            try:
                import jax
                jax.clear_caches()
                jax.extend.backend.clear_backends()
            except Exception:
                pass
            import time as _t
            _t.sleep(5)
    raise last_err



# revision 7
# speedup vs baseline: 2.6653x; 2.6653x over previous
"""Trainium2 Bass kernel for nn_ConvolutionalNMPBlock.

Self-contained: takes full (unsharded) inputs, shards batch across 8
NeuronCores (2 elements each), runs a fused Bass/Tile kernel, gathers.

v2: no DRAM round-trips for the RBF factor build (DVE 32x32 stream
transposes instead), element prep hoisted ahead of compute for cross-
element overlap, A-branch accumulation interleaved with G/exp to keep
the PE busy, PSUM drains and output stores on the otherwise-idle GpSimd
queue.
"""
import numpy as np
import ml_dtypes

BS, N, D = 16, 2048, 256
NCORE = 8
PER = BS // NCORE          # batch elements per core
EPS = 1e-5
NB = N // 128              # 16 row blocks
LC = N // 512              # 4 column chunks of 512
DC = D // 128              # 2 channel blocks
KT = 17                    # conv2 taps
NG = N // 32               # 64 stream-transpose blocks
BF = ml_dtypes.bfloat16

_built = {}                # (use_mask, use_bias) -> compiled nc


def _build(use_mask: bool, use_bias: bool = True, loop_n: int = 1,
           body_reps: int = 1):
    from concourse import bacc, tile
    import concourse.mybir as mybir
    from contextlib import ExitStack

    f32 = mybir.dt.float32
    bf16 = mybir.dt.bfloat16
    AF = mybir.ActivationFunctionType
    OP = mybir.AluOpType

    nc = bacc.Bacc("TRN2", target_bir_lowering=False, debug=False,
                   num_devices=NCORE)

    def din(name, shape, dt=f32):
        return nc.dram_tensor(name, shape, dt, kind="ExternalInput").ap()

    x_d = din("x", (PER, N, D))
    w1_d = din("w1t", (128, DC, DC, 128), bf16)        # [p=kin, kc, mc, m]
    w2_d = din("w2t", (128, DC, KT, DC, 128), bf16)    # [p, kc, tap, mc, m]
    sh1_d = din("sh1", (128, DC))
    sh2_d = din("sh2", (128, DC))
    wmsg_d = din("wmsgt", (128, DC, D), bf16)          # [p, kc, f]
    bmsg_d = din("bmsg", (1, D), bf16)
    wseh_d = din("wseth", (128, DC, 3), bf16)
    wsel_d = din("wsetl", (128, DC, 3), bf16)
    bse_d = din("bse", (1, 3), bf16)
    wih_d = din("wiht", (128, 4, 3 * D), bf16)         # [p, kc, f]
    whh_d = din("whht", (128, DC, 3 * D), bf16)
    brz_d = din("brow_rz", (1, 2 * D), bf16)
    bgin_d = din("brow_gin", (1, D), bf16)
    bghn_d = din("brow_ghn", (1, D), bf16)
    ones_d = din("ones128", (1, 128), bf16)
    ones5_d = din("ones512", (1, 512), bf16)
    ones2n_d = din("ones2n", (2, N), bf16)
    if use_mask:
        mt_d = din("maskt", (PER, N, N), bf16)
    out_d = nc.dram_tensor("out", (PER, N, D), f32, kind="ExternalOutput").ap()

    with tile.TileContext(nc) as tc, ExitStack() as ctx:
        if loop_n > 1:
            ctx.enter_context(tc.For_i(0, loop_n, 1))
        wp = ctx.enter_context(tc.tile_pool(name="wp", bufs=1))
        big = ctx.enter_context(tc.tile_pool(name="big", bufs=1))
        rawp = ctx.enter_context(tc.tile_pool(name="rawp", bufs=1))
        atp = ctx.enter_context(tc.tile_pool(name="atp", bufs=18))
        cvp = ctx.enter_context(tc.tile_pool(name="cvp", bufs=2))
        gtp = ctx.enter_context(tc.tile_pool(name="gtp", bufs=1))
        ps = ctx.enter_context(tc.tile_pool(name="ps", bufs=4, space="PSUM"))
        pa = ctx.enter_context(tc.tile_pool(name="pa", bufs=2, space="PSUM"))

        # ---- persistent weights; s-phase weights first (needed earliest) ----
        wseh = wp.tile([128, DC, 3], bf16, tag="wseh")
        nc.gpsimd.dma_start(wseh[:], wseh_d[:])
        wsel = wp.tile([128, DC, 3], bf16, tag="wsel")
        nc.gpsimd.dma_start(wsel[:], wsel_d[:])
        bse = wp.tile([1, 3], bf16, tag="bse")
        nc.gpsimd.dma_start(bse[:], bse_d[:])
        w1 = wp.tile([128, DC, DC, 128], bf16, tag="w1")
        nc.gpsimd.dma_start(w1[:], w1_d[:])
        sh1 = wp.tile([128, DC], f32, tag="sh1")
        nc.gpsimd.dma_start(sh1[:], sh1_d[:])
        sh2 = wp.tile([128, DC], f32, tag="sh2")
        nc.gpsimd.dma_start(sh2[:], sh2_d[:])
        bmsg = wp.tile([1, D], bf16, tag="bmsg")
        nc.gpsimd.dma_start(bmsg[:], bmsg_d[:])
        ones = wp.tile([1, 128], bf16, tag="ones")
        nc.gpsimd.dma_start(ones[:], ones_d[:])
        ones5 = wp.tile([1, 512], bf16, tag="ones5")
        nc.gpsimd.dma_start(ones5[:], ones5_d[:])
        w2 = wp.tile([128, DC, KT, DC, 128], bf16, tag="w2")
        nc.gpsimd.dma_start(w2[:], w2_d[:])
        wmsg = wp.tile([128, DC, D], bf16, tag="wmsg")
        nc.gpsimd.dma_start(wmsg[:], wmsg_d[:])
        wih = wp.tile([128, 4, 3 * D], bf16, tag="wih")
        nc.gpsimd.dma_start(wih[:], wih_d[:])
        whh = wp.tile([128, DC, 3 * D], bf16, tag="whh")
        nc.gpsimd.dma_start(whh[:], whh_d[:])
        brz = wp.tile([1, 2 * D], bf16, tag="brz")
        nc.gpsimd.dma_start(brz[:], brz_d[:])
        bgin = wp.tile([1, D], bf16, tag="bgin")
        nc.gpsimd.dma_start(bgin[:], bgin_d[:])
        bghn = wp.tile([1, D], bf16, tag="bghn")
        nc.gpsimd.dma_start(bghn[:], bghn_d[:])

        # persistent factor-build staging; zero the never-written pad regions
        # once so the stream transposes read initialized data.
        # st32[c, n] holds s rows (c<3); stw32/sq live per-el below.
        st32 = wp.tile([32, N], f32, tag="st32")
        nc.gpsimd.memset(st32[:], 0.0)
        saw32 = wp.tile([32, NG, 32], bf16, tag="saw32")   # [q, g, r]
        nc.gpsimd.memset(saw32[:, :, 13:32], 0.0)
        sbw32 = wp.tile([32, NG, 32], bf16, tag="sbw32")
        nc.gpsimd.memset(sbw32[:, :, 13:32], 0.0)

        qn = NB // 4
        quarters = [slice(h * qn, (h + 1) * qn) for h in range(4)]

        def prep(el):
            # load x n-major (contiguous), split bf16 hi/lo, and build
            # channel-major copies via the XBAR transpose DMA (2-byte dtype).
            xnf = rawp.tile([128, NB, D], f32, tag="xnf", bufs=2, name=f"xnf{el}")
            xnh = rawp.tile([128, NB, D], bf16, tag="xnh", bufs=1)
            xnl = rawp.tile([128, NB, D], bf16, tag="xnl", bufs=1)
            # layout: xth[p_c, nb, dc, p_n] == xT[dc*128+p_c, nb*128+p_n]
            xth = big.tile([128, NB, DC, 128], bf16, tag="xth", bufs=2,
                           name=f"xth{el}")
            xtl = big.tile([128, NB, DC, 128], bf16, tag="xtl", bufs=2,
                           name=f"xtl{el}")
            for h_, hs in enumerate(quarters):
                eng = nc.sync if h_ % 2 == 0 else nc.scalar
                eng.dma_start(
                    xnf[:, hs, :],
                    x_d[el, h_ * (N // 4):(h_ + 1) * (N // 4), :].rearrange(
                        "(nb p) d -> p nb d", p=128))
            for hs in quarters:
                nc.vector.tensor_copy(xnh[:, hs, :], xnf[:, hs, :])
                nc.vector.scalar_tensor_tensor(xnl[:, hs, :], xnf[:, hs, :],
                                               1.0, xnh[:, hs, :],
                                               OP.mult, OP.subtract)
            for hs in quarters:
                nc.scalar.dma_start_transpose(
                    xth[:, hs].rearrange("p nb dc pn -> p (nb dc) pn"),
                    xnh[:, hs, :])
            for hs in quarters:
                nc.scalar.dma_start_transpose(
                    xtl[:, hs].rearrange("p nb dc pn -> p (nb dc) pn"),
                    xnl[:, hs, :])
            return xnf, xth, xtl

        def compute(el, xnf, xth, xtl):
            # ---- s = x @ w_se.T + b_se (f32-accurate via hi/lo) -> st32 ----
            for c in range(LC):
                pss = ps.tile([3, 512], f32, tag="ps")
                first = True
                for kc in range(DC):
                    for wi_, (wse_, xt_) in enumerate(
                            ((wseh, xth), (wseh, xtl), (wsel, xth))):
                        last = (not use_bias) and kc == DC - 1 and wi_ == 2
                        nc.tensor.matmul(pss[:], wse_[:, kc, :],
                                         xt_[:, 4 * c:4 * (c + 1), kc, :],
                                         start=first, stop=last,
                                         skip_group_check=True)
                        first = False
                if use_bias:
                    nc.tensor.matmul(pss[:], bse[:], ones5[:], start=False,
                                     stop=True, skip_group_check=True)
                nc.vector.tensor_copy(st32[0:3, c * 512:(c + 1) * 512], pss[:])

            # ---- factor build, all in 32-partition wide layout ----
            # stw32[q, g, c] = s[c, 32 g + q] via 32x32 stream transpose
            stw32 = cvp.tile([32, NG, 32], f32, tag="stw32", bufs=1)
            nc.vector.transpose(stw32[:].rearrange("q g c -> q (g c)"),
                                st32[:])
            ssqw = cvp.tile([32, NG, 3], f32, tag="ssqw", bufs=1)
            nc.scalar.activation(ssqw[:], stw32[:, :, 0:3], AF.Square)
            sqw = cvp.tile([32, NG], f32, tag="sqw", bufs=1)
            nc.vector.tensor_tensor(sqw[:], ssqw[:, :, 0], ssqw[:, :, 1], OP.add)
            nc.vector.tensor_tensor(sqw[:], sqw[:], ssqw[:, :, 2], OP.add)

            # 13 factor rows r (SA | SB): 0:3 (2s_hi | s_hi), 3:6 (2s_lo |
            # s_hi), 6:9 (2s_hi | s_lo), 9,10 (1 | -sq_hi, -sq_lo),
            # 11,12 (-sq_hi, -sq_lo | 1)
            nc.vector.tensor_scalar(saw32[:, :, 0:3], stw32[:, :, 0:3],
                                    2.0, None, OP.mult)
            nc.vector.scalar_tensor_tensor(saw32[:, :, 3:6], stw32[:, :, 0:3],
                                           2.0, saw32[:, :, 0:3],
                                           OP.mult, OP.subtract)
            nc.vector.tensor_copy(saw32[:, :, 6:9], saw32[:, :, 0:3])
            nc.vector.memset(saw32[:, :, 9:11], 1.0)
            nc.vector.tensor_scalar(saw32[:, :, 11:12], sqw[:].unsqueeze(2),
                                    -1.0, None, OP.mult)
            nc.vector.scalar_tensor_tensor(saw32[:, :, 12:13],
                                           sqw[:].unsqueeze(2), -1.0,
                                           saw32[:, :, 11:12],
                                           OP.mult, OP.subtract)
            nc.vector.tensor_copy(sbw32[:, :, 0:3], stw32[:, :, 0:3])
            nc.vector.tensor_copy(sbw32[:, :, 3:6], sbw32[:, :, 0:3])
            nc.vector.scalar_tensor_tensor(sbw32[:, :, 6:9], stw32[:, :, 0:3],
                                           1.0, sbw32[:, :, 0:3],
                                           OP.mult, OP.subtract)
            nc.vector.tensor_copy(sbw32[:, :, 9:11], saw32[:, :, 11:13])
            nc.vector.memset(sbw32[:, :, 11:13], 1.0)

            # transpose back: sa[r, 32 g + q] = saw32[q, g, r]; rows 13:32
            # hold zero-propagated pad and are never read.
            sa = cvp.tile([32, N], bf16, tag="sa", bufs=1)
            nc.vector.transpose(sa[:], saw32[:].rearrange("q g r -> q (g r)"))
            sb = cvp.tile([32, N], bf16, tag="sb", bufs=1)
            nc.vector.transpose(sb[:], sbw32[:].rearrange("q g r -> q (g r)"))

            # ---- conv1 (1x1) + bn1 + relu -> h1 (padded by 8 each side) ----
            h1 = big.tile([128, DC, N + 16], bf16, tag="h1", bufs=1)
            nc.vector.memset(h1[:, :, 0:8], 0.0)
            nc.vector.memset(h1[:, :, N + 8:N + 16], 0.0)
            for mc in range(DC):
                for c in range(LC):
                    pc = ps.tile([128, 512], f32, tag="ps")
                    for kc in range(DC):
                        nc.tensor.matmul(pc[:], w1[:, kc, mc, :],
                                         xth[:, 4 * c:4 * (c + 1), kc, :],
                                         start=(kc == 0), stop=(kc == DC - 1),
                                         skip_group_check=True)
                    nc.scalar.activation(h1[:, mc, 8 + c * 512:8 + (c + 1) * 512],
                                         pc[:], AF.Relu, bias=sh1[:, mc:mc + 1])

            # ---- conv2 (17 taps) + bn2 + residual + relu -> x_convT ----
            xcv = big.tile([128, DC, N], bf16, tag="xcv")
            for mc in range(DC):
                for c in range(LC):
                    pc2 = ps.tile([128, 512], f32, tag="ps")
                    first = True
                    for kc in range(DC):
                        for t in range(KT):
                            nc.tensor.matmul(
                                pc2[:], w2[:, kc, t, mc, :],
                                h1[:, kc, c * 512 + t:c * 512 + t + 512],
                                start=first, stop=(kc == DC - 1 and t == KT - 1),
                                skip_group_check=True)
                            first = False
                    tv = cvp.tile([128, 4, 128], f32, tag="cv", bufs=2)
                    nc.vector.tensor_tensor(tv[:], pc2[:].rearrange(
                        "p (a b) -> p a b", b=128),
                        xth[:, 4 * c:4 * (c + 1), mc, :], OP.add)
                    nc.vector.tensor_tensor(tv[:], tv[:],
                                            xtl[:, 4 * c:4 * (c + 1), mc, :],
                                            OP.add)
                    nc.scalar.activation(
                        xcv[:, mc, c * 512:(c + 1) * 512],
                        tv[:].rearrange("p a b -> p (a b)"),
                        AF.Relu, bias=sh2[:, mc:mc + 1])

            # ---- A-branch fused with msg ----
            # chunk 0: msg matmuls interleaved with G matmuls (msg relu on
            # GpSimd so Scalar only runs exp); chunks 1..3: the previous
            # chunk's accumulation matmuls interleave with G/exp so the PE
            # stays busy while exp paces the G PSUM recycle.
            msg = big.tile([128, NB, D], bf16, tag="msg")
            xnm = big.tile([128, DC, N], bf16, tag="xnm")
            at_store = {}

            def g_step(ic, jb):
                pgm = ps.tile([128, 512], f32, tag="ps")
                nc.tensor.matmul(pgm[:], sa[0:13, jb * 128:(jb + 1) * 128],
                                 sb[0:13, ic * 512:(ic + 1) * 512],
                                 start=True, stop=True, skip_group_check=True)
                at = atp.tile([128, 512], bf16, tag="at")
                nc.scalar.activation(at[:], pgm[:], AF.Exp)
                if use_mask:
                    mtt = cvp.tile([128, 512], bf16, tag="mtt")
                    nc.sync.dma_start(mtt[:],
                                      mt_d[el, jb * 128:(jb + 1) * 128,
                                           ic * 512:(ic + 1) * 512])
                    nc.vector.tensor_tensor(at[:], at[:], mtt[:], OP.mult)
                at_store[(ic, jb)] = at

            for jb in range(NB):
                pm = ps.tile([128, 512], f32, tag="ps")
                for kc in range(DC):
                    nc.tensor.matmul(pm[:, 0:D], xth[:, jb, kc, :],
                                     wmsg[:, kc, :], start=(kc == 0),
                                     stop=(not use_bias and kc == DC - 1),
                                     skip_group_check=True)
                if use_bias:
                    nc.tensor.matmul(pm[:, 0:D], ones[:], bmsg[:], start=False,
                                     stop=True, skip_group_check=True)
                nc.vector.tensor_scalar(msg[:, jb, :], pm[:, 0:D], 0.0, None,
                                        OP.max)
                g_step(0, jb)

            for ic in range(1, LC + 1):
                accs = [pa.tile([128, 512], f32, tag=f"acc{mc}", name=f"a{mc}")
                        for mc in range(DC)]
                for jb in range(NB):
                    if ic < LC:
                        g_step(ic, jb)
                    at_prev = at_store.pop((ic - 1, jb))
                    for mc in range(DC):
                        nc.tensor.matmul(accs[mc][:],
                                         msg[:, jb, mc * 128:(mc + 1) * 128],
                                         at_prev[:],
                                         start=(jb == 0), stop=(jb == NB - 1),
                                         skip_group_check=True)
                for mc in range(DC):
                    nc.vector.tensor_copy(
                        xnm[:, mc, (ic - 1) * 512:ic * 512], accs[mc][:])

            # ---- GRU gates (n-major) ----
            for nb in range(NB):
                sl = slice(nb * 128, (nb + 1) * 128)
                prz = ps.tile([128, 512], f32, tag="ps")
                ih_srcs = [xcv[:, 0, sl], xcv[:, 1, sl], xnm[:, 0, sl],
                           xnm[:, 1, sl]]
                hh_srcs = [xth[:, nb, 0, :], xth[:, nb, 1, :]]
                for ci in range(4):
                    nc.tensor.matmul(prz[:], ih_srcs[ci], wih[:, ci, 0:512],
                                     start=(ci == 0), stop=False,
                                     skip_group_check=True)
                for kc in range(DC):
                    nc.tensor.matmul(prz[:], hh_srcs[kc], whh[:, kc, 0:512],
                                     start=False,
                                     stop=(not use_bias and kc == DC - 1),
                                     skip_group_check=True)
                if use_bias:
                    nc.tensor.matmul(prz[:], ones[:], brz[:], start=False,
                                     stop=True, skip_group_check=True)

                # gi_n in cols 0:D, gh_n in cols D:2D of ONE psum tile
                pgg = ps.tile([128, 512], f32, tag="ps")
                for ci in range(4):
                    nc.tensor.matmul(pgg[:, 0:D], ih_srcs[ci],
                                     wih[:, ci, 512:768],
                                     start=(ci == 0),
                                     stop=(not use_bias and ci == 3),
                                     skip_group_check=True)
                if use_bias:
                    nc.tensor.matmul(pgg[:, 0:D], ones[:], bgin[:], start=False,
                                     stop=True, skip_group_check=True)
                for kc in range(DC):
                    nc.tensor.matmul(pgg[:, D:2 * D], hh_srcs[kc],
                                     whh[:, kc, 512:768], start=(kc == 0),
                                     stop=(not use_bias and kc == DC - 1),
                                     skip_group_check=True)
                if use_bias:
                    nc.tensor.matmul(pgg[:, D:2 * D], ones[:], bghn[:],
                                     start=False, stop=True,
                                     skip_group_check=True)

                tr = gtp.tile([128, D], f32, tag="tr")
                nc.scalar.activation(tr[:], prz[:, 0:D], AF.Tanh, scale=0.5)
                tz = gtp.tile([128, D], f32, tag="tz")
                nc.scalar.activation(tz[:], prz[:, D:2 * D], AF.Tanh, scale=0.5)
                nc.vector.tensor_scalar(tz[:], tz[:], 0.5, 0.5, OP.mult, OP.add)
                nc.vector.tensor_scalar(tr[:], tr[:], 0.5, 0.5, OP.mult, OP.add)
                q = gtp.tile([128, D], f32, tag="q")
                nc.vector.tensor_tensor(q[:], tr[:], pgg[:, D:2 * D], OP.mult)
                nc.vector.tensor_tensor(q[:], q[:], pgg[:, 0:D], OP.add)
                nn = gtp.tile([128, D], f32, tag="nn")
                nc.scalar.activation(nn[:], q[:], AF.Tanh)
                dd = gtp.tile([128, D], f32, tag="dd")
                nc.vector.tensor_tensor(dd[:], xnf[:, nb, :], nn[:], OP.subtract)
                nc.vector.tensor_tensor(dd[:], tz[:], dd[:], OP.mult)
                ho = gtp.tile([128, D], f32, tag="ho", bufs=2)
                nc.vector.tensor_tensor(ho[:], nn[:], dd[:], OP.add)
                nc.gpsimd.dma_start(out_d[el, sl, :], ho[:])

        for _ in range(body_reps):
            preps = [prep(el) for el in range(PER)]
            for el in range(PER):
                compute(el, *preps[el])

    nc.compile()
    return nc


def _host_prep(inputs):
    g = {k: np.asarray(v, np.float32) for k, v in inputs.items()}
    sc1 = g["bn1_g"] / np.sqrt(g["bn1_v"] + EPS)
    sh1 = g["bn1_b"] - g["bn1_m"] * sc1
    sc2 = g["bn2_g"] / np.sqrt(g["bn2_v"] + EPS)
    sh2 = g["bn2_b"] - g["bn2_m"] * sc2

    w1p = g["conv1_w"][:, :, 0] * sc1[:, None]          # (O, I)
    w2p = g["conv2_w"] * sc2[:, None, None]             # (O, I, 17)

    def lhsT_pack(w):   # (O, I) -> (128, kc=I/128, mc=O/128, 128): [p,kc,mc,m]
        o, i = w.shape
        return np.ascontiguousarray(np.transpose(
            w.T.reshape(i // 128, 128, o // 128, 128), (1, 0, 2, 3)))

    w1t = lhsT_pack(w1p).astype(BF)
    w2t = np.stack([lhsT_pack(w2p[:, :, t]) for t in range(KT)], axis=2)
    w2t = np.ascontiguousarray(np.transpose(w2t, (0, 1, 2, 3, 4)))  # [p,kc,t,mc,m]
    w2t = w2t.astype(BF)

    def rhs_pack(wt):   # (Kdim, F) -> (128, kc, F)
        k, f = wt.shape
        return np.ascontiguousarray(
            np.transpose(wt.reshape(k // 128, 128, f), (1, 0, 2)))

    wmsgt = rhs_pack(g["w_msg"].T).astype(BF)
    wiht = rhs_pack(g["w_ih"].T).astype(BF)
    whht = rhs_pack(g["w_hh"].T).astype(BF)

    wse_t = g["w_se"].T                                  # (256, 3)
    wse_hi = wse_t.astype(BF)
    wse_lo = (wse_t - wse_hi.astype(np.float32)).astype(BF)
    wseth = rhs_pack(wse_hi.astype(np.float32)).astype(BF)
    wsetl = rhs_pack(wse_lo.astype(np.float32)).astype(BF)

    bih, bhh = g["b_ih"], g["b_hh"]
    feed = {
        "w1t": w1t, "w2t": w2t,
        "sh1": np.ascontiguousarray(sh1.reshape(DC, 128).T.astype(np.float32)),
        "sh2": np.ascontiguousarray(sh2.reshape(DC, 128).T.astype(np.float32)),
        "wmsgt": wmsgt, "bmsg": g["b_msg"].reshape(1, D).astype(BF),
        "wseth": wseth, "wsetl": wsetl,
        "bse": g["b_se"].reshape(1, 3).astype(BF),
        "wiht": wiht, "whht": whht,
        "brow_rz": (bih[:2 * D] + bhh[:2 * D]).reshape(1, 2 * D).astype(BF),
        "brow_gin": bih[2 * D:].reshape(1, D).astype(BF),
        "brow_ghn": bhh[2 * D:].reshape(1, D).astype(BF),
        "ones128": np.ones((1, 128), BF),
        "ones512": np.ones((1, 512), BF),
        "ones2n": np.ones((2, N), BF),
    }
    return g, feed


def make_in_maps(inputs):
    g, feed = _host_prep(inputs)
    x = g["x"]
    mask = g["mask"]
    use_mask = not bool(np.all(mask == 1.0))
    use_bias = not (np.all(g["b_se"] == 0) and np.all(g["b_msg"] == 0)
                    and np.all(g["b_ih"] == 0) and np.all(g["b_hh"] == 0))
    in_maps = []
    for i in range(NCORE):
        m = dict(feed)
        m["x"] = np.ascontiguousarray(x[i * PER:(i + 1) * PER])
        if use_mask:
            m["maskt"] = np.ascontiguousarray(
                mask[i * PER:(i + 1) * PER].transpose(0, 2, 1)).astype(BF)
        in_maps.append(m)
    return in_maps, use_mask, use_bias


def get_nc(use_mask: bool, use_bias: bool = True):
    key = (use_mask, use_bias)
    if key not in _built:
        _built[key] = _build(use_mask, use_bias)
    return _built[key]


def kernel(**inputs) -> np.ndarray:
    in_maps, use_mask, use_bias = make_in_maps(inputs)
    nc = get_nc(use_mask, use_bias)
    from concourse import bass_utils
    last_err = None
    for attempt in range(3):
        try:
            res = bass_utils.run_bass_kernel_spmd(nc, in_maps,
                                                  core_ids=list(range(NCORE)))
            out = np.concatenate([res.results[i]["out"] for i in range(NCORE)],
                                 axis=0)
            return np.ascontiguousarray(out.astype(np.float32))
        except Exception as e:  # wedged device: reset backend and retry
            last_err = e
            try:
                import jax
                jax.clear_caches()
                jax.extend.backend.clear_backends()
            except Exception:
                pass
            import time as _t
            _t.sleep(5)
    raise last_err
